# revision 13
# baseline (speedup 1.0000x reference)
"""Trainium2 Bass kernel for a single-layer transformer block (attention + FFN + 2x LayerNorm).

Shapes (hardcoded): q,k,v [4,4096,128] fp32; w1 [128,512]; w2 [512,128]; out [4,4096,128].
Sharding: 8 cores; core c handles batch c//2, q-rows half c%2 (2048 rows each);
k/v replicated per batch pair. Pure data-parallel SPMD, no collectives.

Key optimizations over the 142us baseline (~20% faster at equal clock):
  - "(p t) d" contiguous-per-partition DRAM views for q/k/v/out: every DMA chunk
    is one contiguous 2KB+ run per partition (128 descriptors) instead of 512
    scattered 512B lines. The induced kpos/row permutation is harmless (softmax
    and attn are permutation-invariant along kpos when k and v share the view;
    the q-row permutation is undone by storing through the same view).
  - DMA schedule in strict consumption order across the three DGE-capable
    queues (sync/scalar/gpsimd); 4-tile chunks throughout (per-dma_start
    latency outweighs finer pacing); every transpose group is woven at a slot
    later than its chunk's arrival, since the in-order PE queue would stall
    the next scores matmul behind a waiting transpose.
  - LN1 ELIDED ENTIRELY in the (runtime-detected) g1=1/be1=0/b1=b2=0 case:
    rstd1 is a positive per-column scale commuting through relu and both FFN
    matmuls, so it scales the whole residual branch uniformly per column --
    which the scale-invariant LN2 cancels EXACTLY. Only the mean is subtracted:
    xr = z - 1(x)mu1 (one K=1 outer-product matmul + one DVE add); this also
    removes ACT (the exp-stream governor) from the LN1 dependency chain.
  - 5-slot exp->attnv skew: the ACT exp of a slot pair (~1 elem/cycle/lane, at
    the table-lookup roofline) slightly exceeds the PE's per-slot matmul time,
    so deep skew keeps the PE immune to exp jitter. The skewed pairs' drain +
    accumulator spill CARRY into the next block's first slots, emitted after
    that block's first scores, so the exp stream never seams at boundaries.
  - spacer pops between cross-engine producers (ACT rstd2 / DVE relu) and the
    PE matmuls that consume them: the in-order PE queue otherwise head-of-line
    blocks the next scores behind a stalled post-op matmul.
  - kernel tail: the last block's posts run as two interleaved half-width
    chains, chain B on the idle score psum pool with a serialized FFN (fits its
    2 slots), relus/psum-drains on the post-exp-idle ACT.
  - LayerNorm2 over partitions via ones-matmuls + K=1 outer-product apply;
    ACT exp/ln/relu/copy pinned to one table set (no swaps); ACT-table warm
    woven between the scalar queue's k DMA issues.
"""

import os
import sys

sys.path.insert(0, "/opt/trn_rl_repo")

from collections import deque
from contextlib import ExitStack

import numpy as np

import concourse.bass as bass  # noqa: F401
from concourse import bacc
import concourse.tile as tile
import concourse.mybir as mybir
from concourse.bass_utils import run_bass_kernel_spmd
from concourse.masks import make_identity

B, S, D, F = 4, 4096, 128, 512
N_CORES = 8
HALF = S // 2          # q rows per core
QBLK = 512             # q rows per block (psum bank free width in fp32)
NQB = HALF // QBLK     # 4 q blocks per core
NKT = S // 128         # 32 kpos tiles
NQT = HALF // 128      # 16 q row tiles
FBLK = F // 128        # 4 FFN chunks
EPS = 1e-5
INV_SQRT_D = float(1.0 / np.sqrt(D))

f32 = mybir.dt.float32
bf16 = mybir.dt.bfloat16
AF = mybir.ActivationFunctionType
ALU = mybir.AluOpType

f32r = mybir.dt.float32r
MMDT = {"bf16": bf16, "f32r": f32r}[os.environ.get("KERNEL_MMDT", "bf16")]


def _emit(nc, tc, ctx, simple):
    q = nc.dram_tensor("q", [HALF, D], f32, kind="ExternalInput")
    k = nc.dram_tensor("k", [S, D], f32, kind="ExternalInput")
    v = nc.dram_tensor("v", [S, D], f32, kind="ExternalInput")
    w1 = nc.dram_tensor("w1", [D, F], f32, kind="ExternalInput")
    b1 = nc.dram_tensor("b1", [F], f32, kind="ExternalInput")
    w2 = nc.dram_tensor("w2", [F, D], f32, kind="ExternalInput")
    b2 = nc.dram_tensor("b2", [D], f32, kind="ExternalInput")
    g1 = nc.dram_tensor("g1", [D], f32, kind="ExternalInput")
    be1 = nc.dram_tensor("be1", [D], f32, kind="ExternalInput")
    g2 = nc.dram_tensor("g2", [D], f32, kind="ExternalInput")
    be2 = nc.dram_tensor("be2", [D], f32, kind="ExternalInput")
    out = nc.dram_tensor("out", [HALF, D], f32, kind="ExternalOutput")

    # ---------------- pools ----------------
    persist = ctx.enter_context(tc.tile_pool(name="persist", bufs=1))
    p_pool = ctx.enter_context(tc.tile_pool(name="p", bufs=6))
    xz_pool = ctx.enter_context(tc.tile_pool(name="xz", bufs=6))
    x_pool = ctx.enter_context(tc.tile_pool(name="x", bufs=4))
    h_pool = ctx.enter_context(tc.tile_pool(name="h", bufs=8))
    st_pool = ctx.enter_context(tc.tile_pool(name="st", bufs=8))
    y_pool = ctx.enter_context(tc.tile_pool(name="y", bufs=6))
    o_pool = ctx.enter_context(tc.tile_pool(name="o", bufs=8))

    score_ps = ctx.enter_context(tc.tile_pool(name="score_ps", bufs=2, space="PSUM"))
    acc_ps = ctx.enter_context(tc.tile_pool(name="acc_ps", bufs=1, space="PSUM"))
    misc_ps = ctx.enter_context(tc.tile_pool(name="misc_ps", bufs=3, space="PSUM"))

    # ---------------- persistent tiles ----------------
    ident_bf = persist.tile([128, 128], MMDT, tag="ident_bf")
    ones_stat = persist.tile([128, 1], MMDT, tag="ones_stat")
    ones_row = persist.tile([1, QBLK], MMDT, tag="ones_row")
    neg_row = persist.tile([1, 128], MMDT, tag="neg_row")
    eps_t = persist.tile([1, 1], f32, tag="eps_t")
    warm_t = persist.tile([1, 1], f32, tag="warm_t")

    # contiguous-per-partition DRAM views: partition p holds rows [p*T,(p+1)*T);
    # every DMA chunk is one contiguous run per partition (128 descriptors).
    # The kpos/row permutation this induces is harmless: softmax+attn are
    # permutation-invariant along kpos when k and v share the view, and the
    # q-row permutation is undone by storing through the same view.
    v_r = v.rearrange("(p t) d -> p t d", p=128)
    k_r = k.rearrange("(p t) d -> p t d", p=128)
    q_r = q.rearrange("(p t) d -> p t d", p=128)
    out_r = out.rearrange("(p t) d -> p t d", p=128)
    v_f = persist.tile([128, NKT, 128], f32, tag="v_f")
    v_sb = persist.tile([128, NKT, 128], MMDT, tag="v_sb")
    k_stage = persist.tile([128, NKT, 128], f32, tag="k_stage")
    q_stage = persist.tile([128, NQT, 128], f32, tag="q_stage")
    # bf16 copies of the stages: transposes must run in bf16 -- fp32 PE
    # transposes are LOW/HIGH dual-pass (races when batched into one psum
    # bank) and f32r ldweights yields all-zero output on hardware
    k8_stage = persist.tile([128, NKT, 128], MMDT, tag="k8_stage")
    q8_stage = persist.tile([128, NQT, 128], MMDT, tag="q8_stage")
    kT = persist.tile([128, S], MMDT, tag="kT")
    qT = persist.tile([128, HALF], MMDT, tag="qT")
    w1_f = persist.tile([128, F], f32, tag="w1_f")
    w2_f = persist.tile([128, FBLK, D], f32, tag="w2_f")
    b1_sb = persist.tile([128, FBLK], f32, tag="b1_sb")
    rows_f = persist.tile([1, 8, 128], f32, tag="rows_f")

    # ---------------- input DMAs: 5 queues, strict consumption order ----------
    # eps on the DVE queue (idle until the first cast) so the ACT warm never
    # waits on gpsimd's memset stream
    nc.vector.memset(eps_t, EPS)
    # sync: q first (first scores matmul), then k chunks interleaved with
    # scalar's, then the late v chunks (gpsimd's queue would deliver them too
    # late), then the deferred q[4:16]
    nc.sync.dma_start(out=q_stage[:, 0:4, :], in_=q_r[:, 0:4, :])
    nc.sync.dma_start(out=k_stage[:, 4:8, :], in_=k_r[:, 4:8, :])
    nc.sync.dma_start(out=k_stage[:, 12:16, :], in_=k_r[:, 12:16, :])
    nc.sync.dma_start(out=k_stage[:, 20:24, :], in_=k_r[:, 20:24, :])
    nc.sync.dma_start(out=v_f[:, 24:28, :], in_=v_r[:, 24:28, :])
    nc.sync.dma_start(out=v_f[:, 28:NKT, :], in_=v_r[:, 28:NKT, :])
    nc.sync.dma_start(out=q_stage[:, 4:10, :], in_=q_r[:, 4:10, :])
    nc.sync.dma_start(out=q_stage[:, 10:NQT, :], in_=q_r[:, 10:NQT, :])
    # scalar: the critical first k chunk split in two so the first cast can
    # start ~1us earlier; warm AFTER all early k issues (exp isn't needed
    # until ~14us, and the warm + table load costs ~2.5us of queue time)
    nc.scalar.dma_start(out=k_stage[:, 0:4, :], in_=k_r[:, 0:4, :])
    nc.scalar.dma_start(out=k_stage[:, 8:12, :], in_=k_r[:, 8:12, :])
    nc.scalar.dma_start(out=k_stage[:, 16:20, :], in_=k_r[:, 16:20, :])
    nc.scalar.activation(warm_t, eps_t, AF.Exp)
    nc.scalar.dma_start(out=k_stage[:, 24:28, :], in_=k_r[:, 24:28, :])
    nc.scalar.dma_start(out=k_stage[:, 28:NKT, :], in_=k_r[:, 28:NKT, :])
    # gpsimd: the early/mid v stream + weights at their deadlines
    nc.gpsimd.dma_start(out=v_f[:, 0:4, :], in_=v_r[:, 0:4, :])
    make_identity(nc, ident_bf)
    nc.gpsimd.memset(ones_stat, 1.0 / D)
    nc.gpsimd.memset(ones_row, 1.0)
    nc.gpsimd.memset(neg_row, -1.0)
    nc.gpsimd.dma_start(out=v_f[:, 4:8, :], in_=v_r[:, 4:8, :])
    nc.gpsimd.dma_start(out=v_f[:, 8:12, :], in_=v_r[:, 8:12, :])
    nc.gpsimd.dma_start(out=v_f[:, 12:16, :], in_=v_r[:, 12:16, :])
    nc.gpsimd.dma_start(out=v_f[:, 16:20, :], in_=v_r[:, 16:20, :])
    nc.gpsimd.dma_start(out=v_f[:, 20:24, :], in_=v_r[:, 20:24, :])
    nc.gpsimd.dma_start(out=w1_f, in_=w1[:, :])
    nc.gpsimd.dma_start(out=w2_f, in_=w2.rearrange("(t p) d -> p t d", p=128))
    if not simple:
        for i, t in enumerate((g1, be1, g2, be2, b2)):
            nc.gpsimd.dma_start(out=rows_f[:, i, :], in_=t.ap().unsqueeze(0))
        nc.gpsimd.dma_start(out=b1_sb, in_=b1.rearrange("(t p) -> p t", p=128))

    # bf16 casts of weights / param rows -- emitted LATE (woven into block 0)
    # so their DMA waits never block earlier DVE work in the in-order queue
    w1_sb = persist.tile([128, F], MMDT, tag="w1_sb")
    w2_sb = persist.tile([128, FBLK, D], MMDT, tag="w2_sb")
    rows = persist.tile([1, 8, 128], MMDT, tag="rows")

    def cast_params():
        if simple:
            return
        # rows: [g1, be1, g2, be2, b2, -g1, -g2] in bf16
        nc.vector.tensor_copy(rows[:, 0:5, :], rows_f[:, 0:5, :])
        nc.vector.tensor_scalar(rows[:, 5, :], rows_f[:, 0, :], -1.0, None, ALU.mult)
        nc.vector.tensor_scalar(rows[:, 6, :], rows_f[:, 2, :], -1.0, None, ALU.mult)

    if simple:
        # g=1, be=0, b1=b2=0: A = 1 (x) rstd, B = -1 (x) (mu*rstd)
        g1_row = g2_row = ones_row[:, 0:128]
        ng1_row = ng2_row = neg_row
        be1_row = be2_row = b2_row = None
    else:
        g1_row, be1_row = rows[:, 0, :], rows[:, 1, :]
        g2_row, be2_row = rows[:, 2, :], rows[:, 3, :]
        b2_row = rows[:, 4, :]
        ng1_row, ng2_row = rows[:, 5, :], rows[:, 6, :]

    # ------- transpose helpers (bf16, batched per psum bank) -------
    def transpose_group(dst, stage8, t0, n):
        """PE-transpose bf16 tiles [t0, t0+n) of stage8 into one psum bank,
        then drain with a single DVE copy into dst columns."""
        grp = misc_ps.tile([128, n, 128], MMDT, tag="misc", name="tgrp")
        for i in range(n):
            nc.tensor.transpose(grp[:, i, :], stage8[:, t0 + i, :], ident_bf)
        nc.vector.tensor_copy(dst[:, t0 * 128 : (t0 + n) * 128], grp)

    def cast_transpose(dst, stage, stage8, t0, n):
        """DVE-cast fp32 stage tiles to bf16, then transpose_group them."""
        nc.vector.tensor_copy(stage8[:, t0 : t0 + n, :], stage[:, t0 : t0 + n, :])
        transpose_group(dst, stage8, t0, n)

    # ---------------- post-attention phase as spreadable op list ----------------
    def layer_norm_T_ops(src_x, src_sq, g_row, ng_row, be_row, dst, pool, ptag):
        """Closures computing LN over the partition dim; src/dst are SBUF APs [128, n].
        Broadcast-free apply: dst = src_x * A + B with
        A = g (x) rstd, B = be (x) 1 - g (x) (mu * rstd), built by K=1 matmuls."""
        ncols = src_x.shape[-1]
        state = {}

        def s1():  # mu (psum row)
            state["mu"] = mu = pool.tile([1, ncols], f32, tag=ptag, name="ps_mu")
            nc.tensor.matmul(mu, ones_stat, src_x)

        def s2():  # E[x^2] (psum row)
            state["ms"] = ms = pool.tile([1, ncols], f32, tag=ptag, name="ps_ms")
            nc.tensor.matmul(ms, ones_stat, src_sq)

        def s3():  # mu -> sbuf st[1]; var = ms - mu^2 -> st[0]; frees mu+ms psum
            state["st"] = st = st_pool.tile([1, 2, ncols], MMDT, tag="st", name="st")
            nc.vector.tensor_copy(st[:, 1, :], state["mu"])
            nc.vector.tensor_tensor(st[:, 0, :], st[:, 1, :], st[:, 1, :], ALU.mult)
            nc.vector.tensor_tensor(st[:, 0, :], state["ms"], st[:, 0, :], ALU.subtract)

        def s4():  # rstd = exp(-0.5*ln(var+eps)) -> st[0] (ACT, one table set;
            # no pow/rsqrt exists outside the ACT tables)
            st = state["st"]
            nc.scalar.activation(st[:, 0, :], st[:, 0, :], AF.Ln, bias=eps_t)
            nc.scalar.activation(st[:, 0, :], st[:, 0, :], AF.Exp, scale=-0.5)

        def s5():  # A = g (x) rstd (psum)
            state["A"] = A = pool.tile([128, ncols], f32, tag=ptag, name="ps_A")
            nc.tensor.matmul(A, g_row, state["st"][:, 0, :])

        def s6():  # mrs = mu*rstd -> st[1] (all sbuf)
            st = state["st"]
            nc.vector.tensor_tensor(st[:, 1, :], st[:, 1, :], st[:, 0, :], ALU.mult)

        def s7():  # B = [be (x) 1] - g (x) mrs (psum)
            state["B"] = Bp = pool.tile([128, ncols], f32, tag=ptag, name="ps_B")
            if be_row is None:
                nc.tensor.matmul(Bp, ng_row, state["st"][:, 1, :])
            else:
                nc.tensor.matmul(Bp, be_row, ones_row[:, :ncols],
                                 start=True, stop=False, skip_group_check=True)
                nc.tensor.matmul(Bp, ng_row, state["st"][:, 1, :],
                                 start=False, stop=True, skip_group_check=True)

        def s8():  # dst = src_x*A + B
            nc.vector.tensor_tensor(dst, src_x, state["A"], ALU.mult)
            nc.vector.tensor_tensor(dst, dst, state["B"], ALU.add)

        # spacer pops between the ACT rstd (s4) and the PE matmuls that consume
        # it (s5/s7): the in-order PE queue otherwise reaches the A-matmul one
        # slot after s4 is queued -- behind an in-flight ~1us exp -- and the
        # stalled A-matmul blocks the next slot's scores behind it
        sp = lambda: None
        return [s1, s2, s3, s4, sp, sp, s5, s6, sp, s7, s8]

    LN_OPS = 11  # len of the list layer_norm_T_ops returns (spacers included)

    def make_post_ops(qb, xz, x, c0, c1, tail=False, pool=misc_ps, ptag="misc",
                      serial_ffn=False, xz0=0):
        """Closures for LN1 + FFN + residual + LN2 + store of columns [c0:c1) of
        block qb. xz ([128,2,*]: x and x^2 in SBUF, starting at block column xz0)
        is produced eagerly at the end of the attention phase so the psum
        accumulator frees early. tail=True rebalances work onto ACT (relu -- it
        is idle once the exp stream has drained)."""
        rows0 = qb * QBLK
        nc_cols = c1 - c0
        cols = slice(c0 - xz0, c1 - xz0)
        state = {}
        ops = []
        sp = lambda: None
        if simple:
            # g1=1, be1=0: LN1's rstd is a positive per-column scale that
            # commutes through relu (relu(s*a)=s*relu(a)) and both FFN matmuls,
            # multiplying the whole residual branch -- which the per-column
            # scale-invariant LN2 then cancels EXACTLY. So never compute it:
            # xr = z - mean(z); ffn_raw = w2.T relu(w1.T xr); LN2(xr+ffn_raw).
            # Kills 2 ACT ops, 2 outer-product matmuls and 2 DVE applies per
            # chain, and removes ACT from the LN1 dependency chain entirely.
            z_ap = xz[:, 0, cols]

            def r1():  # mu1 (psum row)
                state["mu"] = mu = pool.tile([1, nc_cols], f32, tag=ptag, name="ps_mu")
                nc.tensor.matmul(mu, ones_stat, z_ap)

            def r2():  # mu -> sbuf row
                state["mrow"] = mr = st_pool.tile([1, nc_cols], MMDT, tag="st", name="mrow")
                nc.vector.tensor_copy(mr, state["mu"])

            def r3():  # Bn = (-1) (x) mu (psum)
                state["Bn"] = Bn = pool.tile([128, nc_cols], f32, tag=ptag, name="ps_Bn")
                nc.tensor.matmul(Bn, neg_row, state["mrow"])

            def r4():  # xr = z - 1 (x) mu
                nc.vector.tensor_tensor(x[:, cols], z_ap, state["Bn"], ALU.add)

            ops.extend([r1, r2, r3, sp, r4, sp])
        else:
            ops.extend(layer_norm_T_ops(
                xz[:, 0, cols], xz[:, 1, cols], g1_row, ng1_row, be1_row, x[:, cols],
                pool, ptag))

        def ffn_start():
            state["ffn"] = pool.tile([128, nc_cols], f32, tag=ptag, name="ps_ffn")

        ops.append(ffn_start)
        # Emit all h-matmuls+relus BEFORE the w2 accumulation chain: the in-order
        # PE queue then pipelines h(fb+1) behind relu(fb) instead of blocking on
        # the accumulate of fb.
        for fb in range(FBLK):
            def ffn_h(fb=fb):
                ps_h = pool.tile([128, nc_cols], f32, tag=ptag, name="ps_h")
                nc.tensor.matmul(
                    ps_h, w1_sb[:, fb * 128 : (fb + 1) * 128], x[:, cols]
                )
                h_sb = h_pool.tile([128, nc_cols], MMDT, tag="h", name="h_sb")
                if tail:
                    # ACT is idle after the exp stream; relu is in the pinned table
                    if simple:
                        nc.scalar.activation(h_sb, ps_h, AF.Relu)
                    else:
                        nc.scalar.activation(h_sb, ps_h, AF.Relu, bias=b1_sb[:, fb : fb + 1])
                elif simple:
                    nc.vector.tensor_scalar(h_sb, ps_h, 0.0, None, ALU.max)
                else:
                    # relu(x + b1): fused add+max on DVE keeps ACT free for exp
                    nc.vector.tensor_scalar(
                        h_sb, ps_h, b1_sb[:, fb : fb + 1], 0.0, ALU.add, ALU.max
                    )
                state[f"h{fb}"] = h_sb

            ops.append(ffn_h)

        def ffn_acc(fb, stop=False):
            nc.tensor.matmul(
                state["ffn"],
                w2_sb[:, fb, :],
                state[f"h{fb}"],
                start=(fb == 0),
                stop=stop,
                skip_group_check=True,
            )

        def ffn_b2():  # += b2 (x) 1 via K=1 matmul; ends the accumulation group
            nc.tensor.matmul(state["ffn"], b2_row, ones_row[:, :nc_cols],
                             start=False, stop=True, skip_group_check=True)

        if serial_ffn:
            # h(fb) -> relu -> acc(fb) serialized: at most ffn+one h psum tile
            # live, so the chain fits a 2-slot pool. Used by the tail chain that
            # rides the (idle) score pool.
            acc_ops = [lambda: ffn_acc(0), lambda: ffn_acc(1), lambda: ffn_acc(2),
                       lambda: (ffn_acc(3, stop=simple), None if simple else ffn_b2())]
            h_ops = ops[-FBLK:]
            del ops[-FBLK:]
            for hop, aop in zip(h_ops, acc_ops):
                ops.append(hop)
                ops.append(aop)
        else:
            # spacer: let the DVE finish relu(h0)/relu(h1) before the PE's
            # in-order queue reaches the accumulation matmuls
            ops.append(lambda: None)
            ops.append(lambda: (ffn_acc(0), ffn_acc(1)))
            if simple:
                ops.append(lambda: (ffn_acc(2), ffn_acc(3, stop=True)))
            else:
                ops.append(lambda: (ffn_acc(2), ffn_acc(3), ffn_b2()))

        def resid():
            state["zz"] = zz = xz_pool.tile([128, 2, nc_cols], MMDT, tag="xz", name="zz")
            nc.vector.tensor_tensor(zz[:, 0, :], state["ffn"], x[:, cols], ALU.add)
            nc.vector.tensor_tensor(zz[:, 1, :], zz[:, 0, :], zz[:, 0, :], ALU.mult)
            state["y"] = y_pool.tile([128, nc_cols], MMDT, tag="y", name="y")

        ops.append(resid)

        def ln2_first():
            state["ln2"] = layer_norm_T_ops(
                state["zz"][:, 0, :], state["zz"][:, 1, :],
                g2_row, ng2_row, be2_row, state["y"], pool, ptag
            )
            state["ln2"][0]()

        ops.append(ln2_first)
        for i in range(1, LN_OPS):
            ops.append(lambda i=i: state["ln2"][i]())

        nt = nc_cols // 128

        # Batched store: all nt output tiles transposed into ONE psum bank,
        # drained with ONE copy and ONE dma issue (vs nt of each): fewer
        # psum-access latencies on DVE and 4x fewer sync-queue DMA issues.
        def store_transpose(t0, n):
            if "ogrp" not in state:
                state["ogrp"] = pool.tile([128, nt, 128], MMDT, tag=ptag, name="ogrp")
            for t in range(t0, t0 + n):
                nc.tensor.transpose(
                    state["ogrp"][:, t, :], state["y"][:, t * 128 : (t + 1) * 128], ident_bf
                )

        def store_flush():
            o_sb = o_pool.tile([128, nt, 128], f32, tag="o", name="o_sb")
            if tail:
                # ACT is idle once the exp stream drains; its table set keeps
                # `copy`, so the psum drain comes off the busy DVE
                nc.scalar.activation(o_sb, state["ogrp"], AF.Copy)
            else:
                nc.vector.tensor_copy(o_sb, state["ogrp"])
            t0 = (rows0 + c0) // 128
            nc.sync.dma_start(out=out_r[:, t0 : t0 + nt, :], in_=o_sb)

        if nt >= 2:
            ops.append(lambda: store_transpose(0, nt // 2))
            ops.append(lambda: store_transpose(nt // 2, nt - nt // 2))
        else:
            ops.append(lambda: store_transpose(0, nt))
        ops.append(store_flush)
        return ops

    # ---------------- software-pipelined main loop ----------------
    # Per-slot extras: block 0 weaves in the k/q casts + grouped transposes it
    # needs (chunk-paced behind the DMAs); later blocks weave in the previous
    # block's post ops and the next block's q-column transposes.
    def cast_chunk(dst, src, t0, n, engine):
        engine.tensor_copy(dst[:, t0 : t0 + n, :], src[:, t0 : t0 + n, :])

    # prologue: q/k staged in 2-tile steps behind the split first DMA chunks so
    # the first scores matmul can start as soon as kT tile 0 + qT[0:256] exist.
    # NOTHING whose DMA lands later may be emitted before the slot-0 minis: the
    # in-order PE queue would stall them behind it.
    cast_transpose(kT, k_stage, k8_stage, 0, 4)   # slots 0..1
    cast_transpose(qT, q_stage, q8_stage, 0, 4)   # block 0's q columns

    pending = deque()  # post ops of the previous block
    carry = deque()    # previous block's attnv drain + spill, run in the next
                       # block's first slots so the exp stream never seams
    n_slots = NKT // 2
    SKEW = 5  # slots of exp->attnv skew (ACT slack)
    # full-width blocks only: half-width passes for the last block were tried
    # and regressed -- they double the exp instruction count for that block and
    # the ACT queue is the steady-state governor
    passes = [(qb, 0, QBLK) for qb in range(NQB)]
    for qb, col0, ncols in passes:
        rows_sl = slice(qb * QBLK + col0, qb * QBLK + col0 + ncols)
        ps_attn = acc_ps.tile([128, ncols], f32, tag="acc")
        pq = deque()  # pending exp'd pairs awaiting attnv accumulation
        for jp in range(n_slots):
            if qb == 0:
                # All q/k transposes happen during block 0 (its slots carry no
                # post-ops), so transpose psum tiles never contend with the LN
                # tiles in the misc pool. Each group is woven at a slot whose
                # start time is safely AFTER its DMA chunk lands (the in-order
                # PE queue would otherwise stall the next scores behind the
                # transpose's wait); v casts alternate DVE/gpsimd.
                if jp == 0:
                    cast_chunk(v_sb, v_f, 0, 4, nc.vector)
                    cast_chunk(v_sb, v_f, 4, 4, nc.gpsimd)
                elif jp == 2:
                    cast_transpose(kT, k_stage, k8_stage, 4, 4)    # slots 2..3
                elif jp == 3:
                    cast_transpose(kT, k_stage, k8_stage, 8, 4)    # slots 4..5
                    cast_chunk(v_sb, v_f, 8, 4, nc.vector)
                    cast_chunk(v_sb, v_f, 12, 4, nc.gpsimd)
                elif jp == 5:
                    cast_transpose(kT, k_stage, k8_stage, 12, 4)   # slots 6..7
                elif jp == 6:
                    cast_transpose(kT, k_stage, k8_stage, 16, 4)   # slots 8..9
                elif jp == 7:
                    cast_chunk(v_sb, v_f, 16, 4, nc.vector)
                    cast_chunk(v_sb, v_f, 20, 4, nc.gpsimd)
                elif jp == 8:
                    cast_transpose(kT, k_stage, k8_stage, 20, 4)   # slots 10..11
                elif jp == 10:
                    cast_transpose(kT, k_stage, k8_stage, 24, 4)   # slots 12..13
                    cast_chunk(v_sb, v_f, 24, 4, nc.vector)
                    cast_chunk(v_sb, v_f, 28, 4, nc.gpsimd)
                elif jp == 11:
                    cast_transpose(kT, k_stage, k8_stage, 28, 4)   # slots 14..15
                elif jp == 12:
                    cast_transpose(qT, q_stage, q8_stage, 4, 4)    # block 1's q
                elif jp == 13:
                    cast_params()
                elif jp == 14:
                    nc.vector.tensor_copy(w1_sb, w1_f)
            elif qb == 1 and jp == 0:
                nc.vector.tensor_copy(w2_sb, w2_f)
            elif qb == 1 and jp == 2:
                cast_transpose(qT, q_stage, q8_stage, 8, 4)        # block 2's q
            elif qb == 1 and jp == 4:
                cast_transpose(qT, q_stage, q8_stage, 12, 4)       # block 3's q
            ps_s = score_ps.tile([128, 2, ncols], f32, tag="score")
            for hh in range(2):
                jk = 2 * jp + hh
                nc.tensor.matmul(
                    ps_s[:, hh, :], kT[:, jk * 128 : (jk + 1) * 128], qT[:, rows_sl]
                )
            p_sb = p_pool.tile([128, 2, ncols], MMDT, tag="p")
            nc.scalar.activation(p_sb, ps_s, AF.Exp, scale=INV_SQRT_D)
            pq.append((jp, p_sb, 0, ncols))
            # Two-slot skew: accumulate an OLDER pair's P@v so the PE never
            # waits on the ACT exp stream even when it jitters.
            if len(pq) > SKEW:
                jq, old_p, cl0, cl1 = pq.popleft()
                for hh in range(2):
                    jk = 2 * jq + hh
                    nc.tensor.matmul(
                        ps_attn[:, cl0:cl1],
                        v_sb[:, jk, :],
                        old_p[:, hh, :],
                        start=(jk == 0 and cl0 == 0),
                        stop=False,
                        skip_group_check=True,
                    )
            # ~30 post pops per block (spacers included): pop 3/slot so the
            # stream drains by slot ~10 and the late slots are attention-only
            # (PE per-slot 0.86us < ACT 1.11us -- ACT-paced, no starvation).
            # The previous block's carried drain/spill ops go first: their PE
            # matmuls then land AFTER this block's first scores in the
            # in-order queue, keeping ACT fed across the boundary.
            npop_c = 0
            for _ in range(3):
                if carry and npop_c < 2:
                    carry.popleft()()
                    npop_c += 1
                elif pending:
                    pending.popleft()()
        # Package the skewed-pair drains + accumulator spill as closures. For
        # blocks 0..2 they ride `carry` into the next block's first slots (the
        # PE does them while ACT chews the next block's first exps); the last
        # block runs them immediately.
        def make_drain(jq, old_p, cl0, cl1, acc, stop):
            def d():
                for hh in range(2):
                    jk = 2 * jq + hh
                    nc.tensor.matmul(
                        acc[:, cl0:cl1],
                        v_sb[:, jk, :],
                        old_p[:, hh, :],
                        start=False,
                        stop=(stop and hh == 1),
                        skip_group_check=True,
                    )
            return d

        drains = []
        while pq:
            jq, old_p, cl0, cl1 = pq.popleft()
            drains.append(make_drain(jq, old_p, cl0, cl1, ps_attn, not pq))

        def make_spill(qb, acc, ncols):
            def spill():
                # Eagerly spill the attention accumulator so its psum bank
                # frees, and queue the block's post ops.
                xz = xz_pool.tile([128, 2, ncols], MMDT, tag="xz", name="xz")
                nc.vector.tensor_copy(xz[:, 0, :], acc)
                if not simple:
                    # LN1 stats need E[z^2]; the simple path's mean-only
                    # centering doesn't
                    nc.vector.tensor_tensor(
                        xz[:, 1, :], xz[:, 0, :], xz[:, 0, :], ALU.mult)
                x = x_pool.tile([128, ncols], MMDT, tag="x", name="x")
                if qb < NQB - 1:
                    pending.extend(make_post_ops(qb, xz, x, 0, QBLK))
                else:
                    # final block: two half-width chains interleaved; chain B
                    # rides the (now idle) score pool with a serialized FFN
                    # (fits its 2 slots) so the chains never contend for the
                    # same psum ring; tail=True moves the relus to the
                    # post-exp-idle ACT
                    opsA = make_post_ops(qb, xz, x, 0, QBLK // 2, tail=True)
                    opsB = make_post_ops(qb, xz, x, QBLK // 2, QBLK, tail=True,
                                         pool=score_ps, ptag="score",
                                         serial_ffn=True)
                    for i in range(max(len(opsA), len(opsB))):
                        if i < len(opsA):
                            pending.append(opsA[i])
                        if i < len(opsB):
                            pending.append(opsB[i])
            return spill

        if qb < NQB - 1:
            carry.extend(drains)
            carry.append(make_spill(qb, ps_attn, ncols))
        else:
            for d in drains:
                d()
            make_spill(qb, ps_attn, ncols)()
    while carry:
        carry.popleft()()
    while pending:
        pending.popleft()()


def _patched_act_tables(module_arch):
    """Collapse the ACT table choice to the one set containing exp+ln (+relu/copy
    fillers) so the kernel never swaps table sets (~2.7us per swap). Positions are
    preserved because act_func_set_id indexes the original act_info.json order."""
    from concourse.hw_specs import get_activation_tables

    tables = get_activation_tables(module_arch)
    keep = "natural_log_exp_and_others"
    if keep in tables:
        return {
            name: (funcs if name == keep else set())
            for name, funcs in tables.items()
        }
    return tables


def build(simple):
    nc = bacc.Bacc("TRN2", target_bir_lowering=False, debug=False, num_devices=N_CORES)
    with tile.TileContext(nc) as tc:
        with ExitStack() as ctx:
            _emit(nc, tc, ctx, simple)
    import concourse.bacc as bacc_mod

    orig = bacc_mod.get_activation_tables
    bacc_mod.get_activation_tables = _patched_act_tables
    try:
        nc.compile()
    finally:
        bacc_mod.get_activation_tables = orig
    return nc


_CACHE = {}


def _get_nc(simple):
    if simple not in _CACHE:
        _CACHE[simple] = build(simple)
    return _CACHE[simple]


def _is_simple(inputs):
    try:
        return (
            np.allclose(np.asarray(inputs["g1"]), 1.0)
            and np.allclose(np.asarray(inputs["g2"]), 1.0)
            and not np.any(np.asarray(inputs["be1"]))
            and not np.any(np.asarray(inputs["be2"]))
            and not np.any(np.asarray(inputs["b1"]))
            and not np.any(np.asarray(inputs["b2"]))
        )
    except Exception:
        return False


def run(inputs, trace=False, trace_kwargs=None):
    """Run on 8 cores; returns (full_output, BassKernelResults)."""
    nc = _get_nc(_is_simple(inputs))
    q = np.asarray(inputs["q"], dtype=np.float32)
    k = np.asarray(inputs["k"], dtype=np.float32)
    v = np.asarray(inputs["v"], dtype=np.float32)
    flat = {
        name: np.ascontiguousarray(np.asarray(inputs[name], dtype=np.float32))
        for name in ("w1", "b1", "w2", "b2", "g1", "be1", "g2", "be2")
    }
    in_maps = []
    for c in range(N_CORES):
        b, h = divmod(c, 2)
        m = dict(flat)
        m["q"] = np.ascontiguousarray(q[b, h * HALF : (h + 1) * HALF, :])
        m["k"] = np.ascontiguousarray(k[b])
        m["v"] = np.ascontiguousarray(v[b])
        in_maps.append(m)
    res = run_bass_kernel_spmd(
        nc, in_maps, list(range(N_CORES)), trace=trace, **(trace_kwargs or {})
    )
    full = np.empty((B, S, D), dtype=np.float32)
    for c in range(N_CORES):
        b, h = divmod(c, 2)
        full[b, h * HALF : (h + 1) * HALF, :] = res.results[c]["out"]
    return full, res


def kernel(**inputs):
    full, _ = run(inputs, trace=False)
    return full


# revision 15
# speedup vs baseline: 1.0197x; 1.0197x over previous
"""Trainium2 Bass kernel for a single-layer transformer block (attention + FFN + 2x LayerNorm).

Shapes (hardcoded): q,k,v [4,4096,128] fp32; w1 [128,512]; w2 [512,128]; out [4,4096,128].
Sharding: 8 cores; core c handles batch c//2, q-rows half c%2 (2048 rows each);
k/v replicated per batch pair. Pure data-parallel SPMD, no collectives.

Key optimizations over the 142us baseline (~20% faster at equal clock):
  - "(p t) d" contiguous-per-partition DRAM views for q/k/v/out: every DMA chunk
    is one contiguous 2KB+ run per partition (128 descriptors) instead of 512
    scattered 512B lines. The induced kpos/row permutation is harmless (softmax
    and attn are permutation-invariant along kpos when k and v share the view;
    the q-row permutation is undone by storing through the same view).
  - DMA schedule in strict consumption order across the three DGE-capable
    queues (sync/scalar/gpsimd); 4-tile chunks throughout (per-dma_start
    latency outweighs finer pacing); every transpose group is woven at a slot
    later than its chunk's arrival, since the in-order PE queue would stall
    the next scores matmul behind a waiting transpose.
  - LN1 ELIDED ENTIRELY in the (runtime-detected) g1=1/be1=0/b1=b2=0 case:
    rstd1 is a positive per-column scale commuting through relu and both FFN
    matmuls, so it scales the whole residual branch uniformly per column --
    which the scale-invariant LN2 cancels EXACTLY. Only the mean is subtracted:
    xr = z - 1(x)mu1 (one K=1 outer-product matmul + one DVE add); this also
    removes ACT (the exp-stream governor) from the LN1 dependency chain.
  - 6-slot exp->attnv skew: the ACT exp of a slot pair (~1 elem/cycle/lane, at
    the table-lookup roofline) slightly exceeds the PE's per-slot matmul time,
    so deep skew keeps the PE immune to exp jitter. The skewed pairs' drain +
    accumulator spill CARRY into the next block's first slots, emitted after
    that block's first scores, so the exp stream never seams at boundaries.
  - spacer pops between cross-engine producers (ACT rstd2 / DVE relu) and the
    PE matmuls that consume them: the in-order PE queue otherwise head-of-line
    blocks the next scores behind a stalled post-op matmul.
  - kernel tail: the last block's posts run as two interleaved half-width
    chains, chain B on the idle score psum pool with a serialized FFN (fits its
    2 slots), relus/psum-drains on the post-exp-idle ACT.
  - LayerNorm2 over partitions via ones-matmuls + K=1 outer-product apply;
    ACT exp/ln/relu/copy pinned to one table set (no swaps); ACT-table warm
    woven between the scalar queue's k DMA issues.
"""

import os
import sys

sys.path.insert(0, "/opt/trn_rl_repo")

from collections import deque
from contextlib import ExitStack

import numpy as np

import concourse.bass as bass  # noqa: F401
from concourse import bacc
import concourse.tile as tile
import concourse.mybir as mybir
from concourse.bass_utils import run_bass_kernel_spmd
from concourse.masks import make_identity

B, S, D, F = 4, 4096, 128, 512
N_CORES = 8
HALF = S // 2          # q rows per core
QBLK = 512             # q rows per block (psum bank free width in fp32)
NQB = HALF // QBLK     # 4 q blocks per core
NKT = S // 128         # 32 kpos tiles
NQT = HALF // 128      # 16 q row tiles
FBLK = F // 128        # 4 FFN chunks
EPS = 1e-5
INV_SQRT_D = float(1.0 / np.sqrt(D))

f32 = mybir.dt.float32
bf16 = mybir.dt.bfloat16
AF = mybir.ActivationFunctionType
ALU = mybir.AluOpType

f32r = mybir.dt.float32r
MMDT = {"bf16": bf16, "f32r": f32r}[os.environ.get("KERNEL_MMDT", "bf16")]


def _emit(nc, tc, ctx, simple):
    q = nc.dram_tensor("q", [HALF, D], f32, kind="ExternalInput")
    k = nc.dram_tensor("k", [S, D], f32, kind="ExternalInput")
    v = nc.dram_tensor("v", [S, D], f32, kind="ExternalInput")
    w1 = nc.dram_tensor("w1", [D, F], f32, kind="ExternalInput")
    b1 = nc.dram_tensor("b1", [F], f32, kind="ExternalInput")
    w2 = nc.dram_tensor("w2", [F, D], f32, kind="ExternalInput")
    b2 = nc.dram_tensor("b2", [D], f32, kind="ExternalInput")
    g1 = nc.dram_tensor("g1", [D], f32, kind="ExternalInput")
    be1 = nc.dram_tensor("be1", [D], f32, kind="ExternalInput")
    g2 = nc.dram_tensor("g2", [D], f32, kind="ExternalInput")
    be2 = nc.dram_tensor("be2", [D], f32, kind="ExternalInput")
    out = nc.dram_tensor("out", [HALF, D], f32, kind="ExternalOutput")

    # ---------------- pools ----------------
    persist = ctx.enter_context(tc.tile_pool(name="persist", bufs=1))
    p_pool = ctx.enter_context(tc.tile_pool(name="p", bufs=7))
    xz_pool = ctx.enter_context(tc.tile_pool(name="xz", bufs=6))
    x_pool = ctx.enter_context(tc.tile_pool(name="x", bufs=4))
    h_pool = ctx.enter_context(tc.tile_pool(name="h", bufs=8))
    st_pool = ctx.enter_context(tc.tile_pool(name="st", bufs=8))
    y_pool = ctx.enter_context(tc.tile_pool(name="y", bufs=6))
    o_pool = ctx.enter_context(tc.tile_pool(name="o", bufs=8))

    score_ps = ctx.enter_context(tc.tile_pool(name="score_ps", bufs=2, space="PSUM"))
    acc_ps = ctx.enter_context(tc.tile_pool(name="acc_ps", bufs=1, space="PSUM"))
    misc_ps = ctx.enter_context(tc.tile_pool(name="misc_ps", bufs=3, space="PSUM"))

    # ---------------- persistent tiles ----------------
    ident_bf = persist.tile([128, 128], MMDT, tag="ident_bf")
    ones_stat = persist.tile([128, 1], MMDT, tag="ones_stat")
    ones_row = persist.tile([1, QBLK], MMDT, tag="ones_row")
    neg_row = persist.tile([1, 128], MMDT, tag="neg_row")
    eps_t = persist.tile([1, 1], f32, tag="eps_t")
    warm_t = persist.tile([1, 1], f32, tag="warm_t")

    # contiguous-per-partition DRAM views: partition p holds rows [p*T,(p+1)*T);
    # every DMA chunk is one contiguous run per partition (128 descriptors).
    # The kpos/row permutation this induces is harmless: softmax+attn are
    # permutation-invariant along kpos when k and v share the view, and the
    # q-row permutation is undone by storing through the same view.
    v_r = v.rearrange("(p t) d -> p t d", p=128)
    k_r = k.rearrange("(p t) d -> p t d", p=128)
    q_r = q.rearrange("(p t) d -> p t d", p=128)
    out_r = out.rearrange("(p t) d -> p t d", p=128)
    v_f = persist.tile([128, NKT, 128], f32, tag="v_f")
    v_sb = persist.tile([128, NKT, 128], MMDT, tag="v_sb")
    k_stage = persist.tile([128, NKT, 128], f32, tag="k_stage")
    q_stage = persist.tile([128, NQT, 128], f32, tag="q_stage")
    # bf16 copies of the stages: transposes must run in bf16 -- fp32 PE
    # transposes are LOW/HIGH dual-pass (races when batched into one psum
    # bank) and f32r ldweights yields all-zero output on hardware
    k8_stage = persist.tile([128, NKT, 128], MMDT, tag="k8_stage")
    q8_stage = persist.tile([128, NQT, 128], MMDT, tag="q8_stage")
    kT = persist.tile([128, S], MMDT, tag="kT")
    qT = persist.tile([128, HALF], MMDT, tag="qT")
    w1_f = persist.tile([128, F], f32, tag="w1_f")
    w2_f = persist.tile([128, FBLK, D], f32, tag="w2_f")
    b1_sb = persist.tile([128, FBLK], f32, tag="b1_sb")
    rows_f = persist.tile([1, 8, 128], f32, tag="rows_f")

    # ---------------- input DMAs: 3 queues, strict consumption order ----------
    # eps on the DVE queue (idle until the first cast) so the ACT warm never
    # waits on gpsimd's memset stream
    nc.vector.memset(eps_t, EPS)
    # sync: q first (first scores matmul), then k chunks interleaved with
    # scalar's, then the late v chunks (gpsimd's queue would deliver them too
    # late), then the deferred q[4:16]
    nc.sync.dma_start(out=q_stage[:, 0:4, :], in_=q_r[:, 0:4, :])
    nc.sync.dma_start(out=k_stage[:, 4:8, :], in_=k_r[:, 4:8, :])
    nc.sync.dma_start(out=k_stage[:, 12:16, :], in_=k_r[:, 12:16, :])
    nc.sync.dma_start(out=k_stage[:, 20:24, :], in_=k_r[:, 20:24, :])
    nc.sync.dma_start(out=v_f[:, 24:28, :], in_=v_r[:, 24:28, :])
    nc.sync.dma_start(out=v_f[:, 28:NKT, :], in_=v_r[:, 28:NKT, :])
    nc.sync.dma_start(out=q_stage[:, 4:10, :], in_=q_r[:, 4:10, :])
    nc.sync.dma_start(out=q_stage[:, 10:NQT, :], in_=q_r[:, 10:NQT, :])
    # scalar: the critical first k chunk split in two so the first cast can
    # start ~1us earlier; warm AFTER all early k issues (exp isn't needed
    # until ~14us, and the warm + table load costs ~2.5us of queue time)
    nc.scalar.dma_start(out=k_stage[:, 0:4, :], in_=k_r[:, 0:4, :])
    nc.scalar.dma_start(out=k_stage[:, 8:12, :], in_=k_r[:, 8:12, :])
    nc.scalar.dma_start(out=k_stage[:, 16:20, :], in_=k_r[:, 16:20, :])
    nc.scalar.activation(warm_t, eps_t, AF.Exp)
    nc.scalar.dma_start(out=k_stage[:, 24:28, :], in_=k_r[:, 24:28, :])
    nc.scalar.dma_start(out=k_stage[:, 28:NKT, :], in_=k_r[:, 28:NKT, :])
    # gpsimd: the early/mid v stream + weights at their deadlines
    nc.gpsimd.dma_start(out=v_f[:, 0:4, :], in_=v_r[:, 0:4, :])
    make_identity(nc, ident_bf)
    nc.gpsimd.memset(ones_stat, 1.0 / D)
    nc.gpsimd.memset(ones_row, 1.0)
    nc.gpsimd.memset(neg_row, -1.0)
    nc.gpsimd.dma_start(out=v_f[:, 4:8, :], in_=v_r[:, 4:8, :])
    nc.gpsimd.dma_start(out=v_f[:, 8:12, :], in_=v_r[:, 8:12, :])
    nc.gpsimd.dma_start(out=v_f[:, 12:16, :], in_=v_r[:, 12:16, :])
    nc.gpsimd.dma_start(out=v_f[:, 16:20, :], in_=v_r[:, 16:20, :])
    nc.gpsimd.dma_start(out=v_f[:, 20:24, :], in_=v_r[:, 20:24, :])
    nc.gpsimd.dma_start(out=w1_f, in_=w1[:, :])
    nc.gpsimd.dma_start(out=w2_f, in_=w2.rearrange("(t p) d -> p t d", p=128))
    if not simple:
        for i, t in enumerate((g1, be1, g2, be2, b2)):
            nc.gpsimd.dma_start(out=rows_f[:, i, :], in_=t.ap().unsqueeze(0))
        nc.gpsimd.dma_start(out=b1_sb, in_=b1.rearrange("(t p) -> p t", p=128))

    # bf16 casts of weights / param rows -- emitted LATE (woven into block 0)
    # so their DMA waits never block earlier DVE work in the in-order queue
    w1_sb = persist.tile([128, F], MMDT, tag="w1_sb")
    w2_sb = persist.tile([128, FBLK, D], MMDT, tag="w2_sb")
    rows = persist.tile([1, 8, 128], MMDT, tag="rows")

    def cast_params():
        if simple:
            return
        # rows: [g1, be1, g2, be2, b2, -g1, -g2] in bf16
        nc.vector.tensor_copy(rows[:, 0:5, :], rows_f[:, 0:5, :])
        nc.vector.tensor_scalar(rows[:, 5, :], rows_f[:, 0, :], -1.0, None, ALU.mult)
        nc.vector.tensor_scalar(rows[:, 6, :], rows_f[:, 2, :], -1.0, None, ALU.mult)

    if simple:
        # g=1, be=0, b1=b2=0: A = 1 (x) rstd, B = -1 (x) (mu*rstd)
        g1_row = g2_row = ones_row[:, 0:128]
        ng1_row = ng2_row = neg_row
        be1_row = be2_row = b2_row = None
    else:
        g1_row, be1_row = rows[:, 0, :], rows[:, 1, :]
        g2_row, be2_row = rows[:, 2, :], rows[:, 3, :]
        b2_row = rows[:, 4, :]
        ng1_row, ng2_row = rows[:, 5, :], rows[:, 6, :]

    # ------- transpose helpers (bf16, batched per psum bank) -------
    def transpose_group(dst, stage8, t0, n):
        """PE-transpose bf16 tiles [t0, t0+n) of stage8 into one psum bank,
        then drain with a single DVE copy into dst columns."""
        grp = misc_ps.tile([128, n, 128], MMDT, tag="misc", name="tgrp")
        for i in range(n):
            nc.tensor.transpose(grp[:, i, :], stage8[:, t0 + i, :], ident_bf)
        nc.vector.tensor_copy(dst[:, t0 * 128 : (t0 + n) * 128], grp)

    def cast_transpose(dst, stage, stage8, t0, n):
        """DVE-cast fp32 stage tiles to bf16, then transpose_group them."""
        nc.vector.tensor_copy(stage8[:, t0 : t0 + n, :], stage[:, t0 : t0 + n, :])
        transpose_group(dst, stage8, t0, n)

    # ---------------- post-attention phase as spreadable op list ----------------
    def layer_norm_T_ops(src_x, src_sq, g_row, ng_row, be_row, dst, pool, ptag):
        """Closures computing LN over the partition dim; src/dst are SBUF APs [128, n].
        Broadcast-free apply: dst = src_x * A + B with
        A = g (x) rstd, B = be (x) 1 - g (x) (mu * rstd), built by K=1 matmuls."""
        ncols = src_x.shape[-1]
        state = {}

        def s1():  # mu (psum row)
            state["mu"] = mu = pool.tile([1, ncols], f32, tag=ptag, name="ps_mu")
            nc.tensor.matmul(mu, ones_stat, src_x)

        def s2():  # E[x^2] (psum row)
            state["ms"] = ms = pool.tile([1, ncols], f32, tag=ptag, name="ps_ms")
            nc.tensor.matmul(ms, ones_stat, src_sq)

        def s3():  # mu -> sbuf st[1]; var = ms - mu^2 -> st[0]; frees mu+ms psum
            state["st"] = st = st_pool.tile([1, 2, ncols], MMDT, tag="st", name="st")
            nc.vector.tensor_copy(st[:, 1, :], state["mu"])
            nc.vector.tensor_tensor(st[:, 0, :], st[:, 1, :], st[:, 1, :], ALU.mult)
            nc.vector.tensor_tensor(st[:, 0, :], state["ms"], st[:, 0, :], ALU.subtract)

        def s4():  # rstd = exp(-0.5*ln(var+eps)) -> st[0] (ACT, one table set;
            # no pow/rsqrt exists outside the ACT tables)
            st = state["st"]
            nc.scalar.activation(st[:, 0, :], st[:, 0, :], AF.Ln, bias=eps_t)
            nc.scalar.activation(st[:, 0, :], st[:, 0, :], AF.Exp, scale=-0.5)

        def s5():  # A = g (x) rstd (psum)
            state["A"] = A = pool.tile([128, ncols], f32, tag=ptag, name="ps_A")
            nc.tensor.matmul(A, g_row, state["st"][:, 0, :])

        def s6():  # mrs = mu*rstd -> st[1] (all sbuf)
            st = state["st"]
            nc.vector.tensor_tensor(st[:, 1, :], st[:, 1, :], st[:, 0, :], ALU.mult)

        def s7():  # B = [be (x) 1] - g (x) mrs (psum)
            state["B"] = Bp = pool.tile([128, ncols], f32, tag=ptag, name="ps_B")
            if be_row is None:
                nc.tensor.matmul(Bp, ng_row, state["st"][:, 1, :])
            else:
                nc.tensor.matmul(Bp, be_row, ones_row[:, :ncols],
                                 start=True, stop=False, skip_group_check=True)
                nc.tensor.matmul(Bp, ng_row, state["st"][:, 1, :],
                                 start=False, stop=True, skip_group_check=True)

        def s8():  # dst = src_x*A + B
            nc.vector.tensor_tensor(dst, src_x, state["A"], ALU.mult)
            nc.vector.tensor_tensor(dst, dst, state["B"], ALU.add)

        # spacer pops between the ACT rstd (s4) and the PE matmuls that consume
        # it (s5/s7): the in-order PE queue otherwise reaches the A-matmul one
        # slot after s4 is queued -- behind an in-flight ~1us exp -- and the
        # stalled A-matmul blocks the next slot's scores behind it
        sp = lambda: None
        return [s1, s2, s3, s4, sp, sp, s5, s6, sp, s7, s8]

    LN_OPS = 11  # len of the list layer_norm_T_ops returns (spacers included)

    def make_post_ops(qb, xz, x, c0, c1, tail=False, pool=misc_ps, ptag="misc",
                      serial_ffn=False, xz0=0):
        """Closures for LN1 + FFN + residual + LN2 + store of columns [c0:c1) of
        block qb. xz ([128,2,*]: x and x^2 in SBUF, starting at block column xz0)
        is produced eagerly at the end of the attention phase so the psum
        accumulator frees early. tail=True rebalances work onto ACT (relu -- it
        is idle once the exp stream has drained)."""
        rows0 = qb * QBLK
        nc_cols = c1 - c0
        cols = slice(c0 - xz0, c1 - xz0)
        state = {}
        ops = []
        sp = lambda: None
        if simple:
            # g1=1, be1=0: LN1's rstd is a positive per-column scale that
            # commutes through relu (relu(s*a)=s*relu(a)) and both FFN matmuls,
            # multiplying the whole residual branch -- which the per-column
            # scale-invariant LN2 then cancels EXACTLY. So never compute it:
            # xr = z - mean(z); ffn_raw = w2.T relu(w1.T xr); LN2(xr+ffn_raw).
            # Kills 2 ACT ops, 2 outer-product matmuls and 2 DVE applies per
            # chain, and removes ACT from the LN1 dependency chain entirely.
            z_ap = xz[:, 0, cols]

            def r1():  # mu1 (psum row)
                state["mu"] = mu = pool.tile([1, nc_cols], f32, tag=ptag, name="ps_mu")
                nc.tensor.matmul(mu, ones_stat, z_ap)

            def r2():  # mu -> sbuf row
                state["mrow"] = mr = st_pool.tile([1, nc_cols], MMDT, tag="st", name="mrow")
                nc.vector.tensor_copy(mr, state["mu"])

            def r3():  # Bn = (-1) (x) mu (psum)
                state["Bn"] = Bn = pool.tile([128, nc_cols], f32, tag=ptag, name="ps_Bn")
                nc.tensor.matmul(Bn, neg_row, state["mrow"])

            def r4():  # xr = z - 1 (x) mu
                nc.vector.tensor_tensor(x[:, cols], z_ap, state["Bn"], ALU.add)

            ops.extend([r1, r2, r3, sp, r4, sp])
        else:
            ops.extend(layer_norm_T_ops(
                xz[:, 0, cols], xz[:, 1, cols], g1_row, ng1_row, be1_row, x[:, cols],
                pool, ptag))

        def ffn_start():
            state["ffn"] = pool.tile([128, nc_cols], f32, tag=ptag, name="ps_ffn")

        ops.append(ffn_start)
        # Emit all h-matmuls+relus BEFORE the w2 accumulation chain: the in-order
        # PE queue then pipelines h(fb+1) behind relu(fb) instead of blocking on
        # the accumulate of fb.
        for fb in range(FBLK):
            def ffn_h(fb=fb):
                ps_h = pool.tile([128, nc_cols], f32, tag=ptag, name="ps_h")
                nc.tensor.matmul(
                    ps_h, w1_sb[:, fb * 128 : (fb + 1) * 128], x[:, cols]
                )
                h_sb = h_pool.tile([128, nc_cols], MMDT, tag="h", name="h_sb")
                if tail:
                    # ACT is idle after the exp stream; relu is in the pinned table
                    if simple:
                        nc.scalar.activation(h_sb, ps_h, AF.Relu)
                    else:
                        nc.scalar.activation(h_sb, ps_h, AF.Relu, bias=b1_sb[:, fb : fb + 1])
                elif simple:
                    nc.vector.tensor_scalar(h_sb, ps_h, 0.0, None, ALU.max)
                else:
                    # relu(x + b1): fused add+max on DVE keeps ACT free for exp
                    nc.vector.tensor_scalar(
                        h_sb, ps_h, b1_sb[:, fb : fb + 1], 0.0, ALU.add, ALU.max
                    )
                state[f"h{fb}"] = h_sb

            ops.append(ffn_h)

        def ffn_acc(fb, stop=False):
            nc.tensor.matmul(
                state["ffn"],
                w2_sb[:, fb, :],
                state[f"h{fb}"],
                start=(fb == 0),
                stop=stop,
                skip_group_check=True,
            )

        def ffn_b2():  # += b2 (x) 1 via K=1 matmul; ends the accumulation group
            nc.tensor.matmul(state["ffn"], b2_row, ones_row[:, :nc_cols],
                             start=False, stop=True, skip_group_check=True)

        if serial_ffn:
            # h(fb) -> relu -> acc(fb) serialized: at most ffn+one h psum tile
            # live, so the chain fits a 2-slot pool. Used by the tail chain that
            # rides the (idle) score pool.
            acc_ops = [lambda: ffn_acc(0), lambda: ffn_acc(1), lambda: ffn_acc(2),
                       lambda: (ffn_acc(3, stop=simple), None if simple else ffn_b2())]
            h_ops = ops[-FBLK:]
            del ops[-FBLK:]
            for hop, aop in zip(h_ops, acc_ops):
                ops.append(hop)
                ops.append(aop)
        else:
            # spacer: let the DVE finish relu(h0)/relu(h1) before the PE's
            # in-order queue reaches the accumulation matmuls
            ops.append(lambda: None)
            ops.append(lambda: (ffn_acc(0), ffn_acc(1)))
            if simple:
                ops.append(lambda: (ffn_acc(2), ffn_acc(3, stop=True)))
            else:
                ops.append(lambda: (ffn_acc(2), ffn_acc(3), ffn_b2()))

        def resid():
            state["zz"] = zz = xz_pool.tile([128, 2, nc_cols], MMDT, tag="xz", name="zz")
            nc.vector.tensor_tensor(zz[:, 0, :], state["ffn"], x[:, cols], ALU.add)
            nc.vector.tensor_tensor(zz[:, 1, :], zz[:, 0, :], zz[:, 0, :], ALU.mult)
            state["y"] = y_pool.tile([128, nc_cols], MMDT, tag="y", name="y")

        ops.append(resid)

        def ln2_first():
            state["ln2"] = layer_norm_T_ops(
                state["zz"][:, 0, :], state["zz"][:, 1, :],
                g2_row, ng2_row, be2_row, state["y"], pool, ptag
            )
            state["ln2"][0]()

        ops.append(ln2_first)
        for i in range(1, LN_OPS):
            ops.append(lambda i=i: state["ln2"][i]())

        nt = nc_cols // 128

        # Batched store: all nt output tiles transposed into ONE psum bank,
        # drained with ONE copy and ONE dma issue (vs nt of each): fewer
        # psum-access latencies on DVE and 4x fewer sync-queue DMA issues.
        def store_transpose(t0, n):
            if "ogrp" not in state:
                state["ogrp"] = pool.tile([128, nt, 128], MMDT, tag=ptag, name="ogrp")
            for t in range(t0, t0 + n):
                nc.tensor.transpose(
                    state["ogrp"][:, t, :], state["y"][:, t * 128 : (t + 1) * 128], ident_bf
                )

        def store_flush():
            o_sb = o_pool.tile([128, nt, 128], f32, tag="o", name="o_sb")
            if tail:
                # ACT is idle once the exp stream drains; its table set keeps
                # `copy`, so the psum drain comes off the busy DVE
                nc.scalar.activation(o_sb, state["ogrp"], AF.Copy)
            else:
                nc.vector.tensor_copy(o_sb, state["ogrp"])
            t0 = (rows0 + c0) // 128
            nc.sync.dma_start(out=out_r[:, t0 : t0 + nt, :], in_=o_sb)

        if nt >= 2:
            ops.append(lambda: store_transpose(0, nt // 2))
            ops.append(lambda: store_transpose(nt // 2, nt - nt // 2))
        else:
            ops.append(lambda: store_transpose(0, nt))
        ops.append(store_flush)
        return ops

    # ---------------- software-pipelined main loop ----------------
    # Per-slot extras: block 0 weaves in the k/q casts + grouped transposes it
    # needs (chunk-paced behind the DMAs); later blocks weave in the previous
    # block's post ops and the next block's q-column transposes.
    def cast_chunk(dst, src, t0, n, engine):
        engine.tensor_copy(dst[:, t0 : t0 + n, :], src[:, t0 : t0 + n, :])

    # prologue: q/k staged in 2-tile steps behind the split first DMA chunks so
    # the first scores matmul can start as soon as kT tile 0 + qT[0:256] exist.
    # NOTHING whose DMA lands later may be emitted before the slot-0 minis: the
    # in-order PE queue would stall them behind it.
    cast_transpose(kT, k_stage, k8_stage, 0, 4)   # slots 0..1
    cast_transpose(qT, q_stage, q8_stage, 0, 4)   # block 0's q columns

    pending = deque()  # post ops of the previous block
    carry = deque()    # previous block's attnv drain + spill, run in the next
                       # block's first slots so the exp stream never seams
    n_slots = NKT // 2
    SKEW = 6  # slots of exp->attnv skew (ACT slack)
    # full-width blocks only: half-width passes for the last block were tried
    # and regressed -- they double the exp instruction count for that block and
    # the ACT queue is the steady-state governor
    passes = [(qb, 0, QBLK) for qb in range(NQB)]
    for qb, col0, ncols in passes:
        rows_sl = slice(qb * QBLK + col0, qb * QBLK + col0 + ncols)
        ps_attn = acc_ps.tile([128, ncols], f32, tag="acc")
        pq = deque()  # pending exp'd pairs awaiting attnv accumulation
        for jp in range(n_slots):
            if qb == 0:
                # All q/k transposes happen during block 0 (its slots carry no
                # post-ops), so transpose psum tiles never contend with the LN
                # tiles in the misc pool. Each group is woven at a slot whose
                # start time is safely AFTER its DMA chunk lands (the in-order
                # PE queue would otherwise stall the next scores behind the
                # transpose's wait); v casts alternate DVE/gpsimd.
                if jp == 0:
                    cast_chunk(v_sb, v_f, 0, 4, nc.vector)
                    cast_chunk(v_sb, v_f, 4, 4, nc.gpsimd)
                elif jp == 2:
                    cast_transpose(kT, k_stage, k8_stage, 4, 4)    # slots 2..3
                elif jp == 3:
                    cast_transpose(kT, k_stage, k8_stage, 8, 4)    # slots 4..5
                    cast_chunk(v_sb, v_f, 8, 4, nc.vector)
                    cast_chunk(v_sb, v_f, 12, 4, nc.gpsimd)
                elif jp == 5:
                    cast_transpose(kT, k_stage, k8_stage, 12, 4)   # slots 6..7
                elif jp == 6:
                    cast_transpose(kT, k_stage, k8_stage, 16, 4)   # slots 8..9
                elif jp == 7:
                    cast_chunk(v_sb, v_f, 16, 4, nc.vector)
                    cast_chunk(v_sb, v_f, 20, 4, nc.gpsimd)
                elif jp == 8:
                    cast_transpose(kT, k_stage, k8_stage, 20, 4)   # slots 10..11
                elif jp == 10:
                    cast_transpose(kT, k_stage, k8_stage, 24, 4)   # slots 12..13
                    cast_chunk(v_sb, v_f, 24, 4, nc.vector)
                    cast_chunk(v_sb, v_f, 28, 4, nc.gpsimd)
                elif jp == 11:
                    cast_transpose(kT, k_stage, k8_stage, 28, 4)   # slots 14..15
                elif jp == 12:
                    cast_transpose(qT, q_stage, q8_stage, 4, 4)    # block 1's q
                elif jp == 13:
                    cast_params()
                elif jp == 14:
                    nc.vector.tensor_copy(w1_sb, w1_f)
            elif qb == 1 and jp == 0:
                nc.vector.tensor_copy(w2_sb, w2_f)
            elif qb == 1 and jp == 2:
                cast_transpose(qT, q_stage, q8_stage, 8, 4)        # block 2's q
            elif qb == 1 and jp == 4:
                cast_transpose(qT, q_stage, q8_stage, 12, 4)       # block 3's q
            ps_s = score_ps.tile([128, 2, ncols], f32, tag="score")
            for hh in range(2):
                jk = 2 * jp + hh
                nc.tensor.matmul(
                    ps_s[:, hh, :], kT[:, jk * 128 : (jk + 1) * 128], qT[:, rows_sl]
                )
            p_sb = p_pool.tile([128, 2, ncols], MMDT, tag="p")
            nc.scalar.activation(p_sb, ps_s, AF.Exp, scale=INV_SQRT_D)
            pq.append((jp, p_sb, 0, ncols))
            # Two-slot skew: accumulate an OLDER pair's P@v so the PE never
            # waits on the ACT exp stream even when it jitters.
            if len(pq) > SKEW:
                jq, old_p, cl0, cl1 = pq.popleft()
                for hh in range(2):
                    jk = 2 * jq + hh
                    nc.tensor.matmul(
                        ps_attn[:, cl0:cl1],
                        v_sb[:, jk, :],
                        old_p[:, hh, :],
                        start=(jk == 0 and cl0 == 0),
                        stop=False,
                        skip_group_check=True,
                    )
            # ~30 post pops per block (spacers included): pop 3/slot so the
            # stream drains by slot ~10 and the late slots are attention-only
            # (PE per-slot 0.86us < ACT 1.11us -- ACT-paced, no starvation).
            # The previous block's carried drain/spill ops go first: their PE
            # matmuls then land AFTER this block's first scores in the
            # in-order queue, keeping ACT fed across the boundary.
            npop_c = 0
            for _ in range(3):
                if carry and npop_c < 2:
                    carry.popleft()()
                    npop_c += 1
                elif pending:
                    pending.popleft()()
        # Package the skewed-pair drains + accumulator spill as closures. For
        # blocks 0..2 they ride `carry` into the next block's first slots (the
        # PE does them while ACT chews the next block's first exps); the last
        # block runs them immediately.
        def make_drain(jq, old_p, cl0, cl1, acc, stop):
            def d():
                for hh in range(2):
                    jk = 2 * jq + hh
                    nc.tensor.matmul(
                        acc[:, cl0:cl1],
                        v_sb[:, jk, :],
                        old_p[:, hh, :],
                        start=False,
                        stop=(stop and hh == 1),
                        skip_group_check=True,
                    )
            return d

        drains = []
        while pq:
            jq, old_p, cl0, cl1 = pq.popleft()
            drains.append(make_drain(jq, old_p, cl0, cl1, ps_attn, not pq))

        def make_spill(qb, acc, ncols):
            def spill():
                # Eagerly spill the attention accumulator so its psum bank
                # frees, and queue the block's post ops.
                xz = xz_pool.tile([128, 2, ncols], MMDT, tag="xz", name="xz")
                nc.vector.tensor_copy(xz[:, 0, :], acc)
                if not simple:
                    # LN1 stats need E[z^2]; the simple path's mean-only
                    # centering doesn't
                    nc.vector.tensor_tensor(
                        xz[:, 1, :], xz[:, 0, :], xz[:, 0, :], ALU.mult)
                x = x_pool.tile([128, ncols], MMDT, tag="x", name="x")
                if qb < NQB - 1:
                    pending.extend(make_post_ops(qb, xz, x, 0, QBLK))
                else:
                    # final block: two half-width chains interleaved; chain B
                    # rides the (now idle) score pool with a serialized FFN
                    # (fits its 2 slots) so the chains never contend for the
                    # same psum ring; tail=True moves the relus to the
                    # post-exp-idle ACT
                    opsA = make_post_ops(qb, xz, x, 0, QBLK // 2, tail=True)
                    opsB = make_post_ops(qb, xz, x, QBLK // 2, QBLK, tail=True,
                                         pool=score_ps, ptag="score",
                                         serial_ffn=True)
                    for i in range(max(len(opsA), len(opsB))):
                        if i < len(opsA):
                            pending.append(opsA[i])
                        if i < len(opsB):
                            pending.append(opsB[i])
            return spill

        if qb < NQB - 1:
            carry.extend(drains)
            carry.append(make_spill(qb, ps_attn, ncols))
        else:
            for d in drains:
                d()
            make_spill(qb, ps_attn, ncols)()
    while carry:
        carry.popleft()()
    while pending:
        pending.popleft()()


def _patched_act_tables(module_arch):
    """Collapse the ACT table choice to the one set containing exp+ln (+relu/copy
    fillers) so the kernel never swaps table sets (~2.7us per swap). Positions are
    preserved because act_func_set_id indexes the original act_info.json order."""
    from concourse.hw_specs import get_activation_tables

    tables = get_activation_tables(module_arch)
    keep = "natural_log_exp_and_others"
    if keep in tables:
        return {
            name: (funcs if name == keep else set())
            for name, funcs in tables.items()
        }
    return tables


def build(simple):
    nc = bacc.Bacc("TRN2", target_bir_lowering=False, debug=False, num_devices=N_CORES)
    with tile.TileContext(nc) as tc:
        with ExitStack() as ctx:
            _emit(nc, tc, ctx, simple)
    import concourse.bacc as bacc_mod

    orig = bacc_mod.get_activation_tables
    bacc_mod.get_activation_tables = _patched_act_tables
    try:
        nc.compile()
    finally:
        bacc_mod.get_activation_tables = orig
    return nc


_CACHE = {}


def _get_nc(simple):
    if simple not in _CACHE:
        _CACHE[simple] = build(simple)
    return _CACHE[simple]


def _is_simple(inputs):
    try:
        return (
            np.allclose(np.asarray(inputs["g1"]), 1.0)
            and np.allclose(np.asarray(inputs["g2"]), 1.0)
            and not np.any(np.asarray(inputs["be1"]))
            and not np.any(np.asarray(inputs["be2"]))
            and not np.any(np.asarray(inputs["b1"]))
            and not np.any(np.asarray(inputs["b2"]))
        )
    except Exception:
        return False


def run(inputs, trace=False, trace_kwargs=None):
    """Run on 8 cores; returns (full_output, BassKernelResults)."""
    nc = _get_nc(_is_simple(inputs))
    q = np.asarray(inputs["q"], dtype=np.float32)
    k = np.asarray(inputs["k"], dtype=np.float32)
    v = np.asarray(inputs["v"], dtype=np.float32)
    flat = {
        name: np.ascontiguousarray(np.asarray(inputs[name], dtype=np.float32))
        for name in ("w1", "b1", "w2", "b2", "g1", "be1", "g2", "be2")
    }
    in_maps = []
    for c in range(N_CORES):
        b, h = divmod(c, 2)
        m = dict(flat)
        m["q"] = np.ascontiguousarray(q[b, h * HALF : (h + 1) * HALF, :])
        m["k"] = np.ascontiguousarray(k[b])
        m["v"] = np.ascontiguousarray(v[b])
        in_maps.append(m)
    res = run_bass_kernel_spmd(
        nc, in_maps, list(range(N_CORES)), trace=trace, **(trace_kwargs or {})
    )
    full = np.empty((B, S, D), dtype=np.float32)
    for c in range(N_CORES):
        b, h = divmod(c, 2)
        full[b, h * HALF : (h + 1) * HALF, :] = res.results[c]["out"]
    return full, res


def kernel(**inputs):
    full, _ = run(inputs, trace=False)
    return full
